# revision 1
# baseline (speedup 1.0000x reference)
"""Trainium2 Bass kernel for nn_ActorBatchNet (Set2Set + torsion MLP).

Full inputs in, full output out. Internally: data-parallel over graphs,
8 NeuronCores x 256 graphs. Small weights replicated; x replicated for the
cross-graph node gather; everything else sharded along the graph axis.
"""

import sys

for _p in ("/opt/trn_rl_repo", "/root/.axon_site/_ro/trn_rl_repo"):
    if _p not in sys.path:
        sys.path.insert(0, _p)

import numpy as np

import concourse.bass as bass
import concourse.bacc as bacc
import concourse.mybir as mybir
import concourse.tile as tile
from concourse.tile import TileContext
from concourse.bass_utils import run_bass_kernel_spmd

F32 = mybir.dt.float32
I32 = mybir.dt.int32
AF = mybir.ActivationFunctionType

# Problem constants (fixed by the reference model)
G = 2048
NODES_PER_G = 64
DIM = 128
TORS_PER_G = 32
ACTD = 36
STEPS = 6
MAX_T = 32
NC = 8                      # cores
NG = G // NC                # graphs per core = 256
NN = NG * NODES_PER_G       # nodes per core  = 16384
NT = NG * TORS_PER_G        # torsions per core = 8192
NTILE = NN // 128           # node tiles per core = 128
TTILE = NT // 128           # torsion tiles per core = 64
P = 128

LAST = None  # BassKernelResults of the most recent run (for test harness)
_CACHED = None


def build_bass():
    nc = bacc.Bacc("TRN2", target_bir_lowering=False, debug=False)

    # ---- DRAM parameters (per core) ----
    xloc = nc.declare_dram_parameter("xloc", [NN, DIM], F32, isOutput=False)
    xlocT = nc.declare_dram_parameter("xlocT", [DIM, NN], F32, isOutput=False)
    xfull = nc.declare_dram_parameter("xfull", [G * NODES_PER_G, DIM], F32, isOutput=False)
    idx = nc.declare_dram_parameter("idx", [P, 4 * TTILE], I32, isOutput=False)
    wA = nc.declare_dram_parameter("wA", [DIM, 4 * DIM], F32, isOutput=False)      # (w_ih[:, :128]+w_hh).T
    wB = nc.declare_dram_parameter("wB", [DIM, 4 * DIM], F32, isOutput=False)      # w_ih[:, 128:].T
    bsum4 = nc.declare_dram_parameter("bsum4", [P, 4], F32, isOutput=False)        # (b_ih+b_hh) col-per-gate
    fcwTq = nc.declare_dram_parameter("fcwTq", [DIM, DIM], F32, isOutput=False)    # fc_w.T[:128]
    fcwTr = nc.declare_dram_parameter("fcwTr", [DIM, DIM], F32, isOutput=False)    # fc_w.T[128:]
    fcb = nc.declare_dram_parameter("fcb", [P, 1], F32, isOutput=False)
    w1Tge = nc.declare_dram_parameter("w1Tge", [DIM, DIM], F32, isOutput=False)    # mlp_w1.T[:128]
    w1Tx = nc.declare_dram_parameter("w1Tx", [4 * DIM, DIM], F32, isOutput=False)  # mlp_w1.T[128:]
    b1 = nc.declare_dram_parameter("b1", [P, 1], F32, isOutput=False)
    w2T = nc.declare_dram_parameter("w2T", [DIM, ACTD], F32, isOutput=False)       # mlp_w2.T
    b2t = nc.declare_dram_parameter("b2t", [P, ACTD], F32, isOutput=False)         # tiled bias
    maskc = nc.declare_dram_parameter("maskc", [P, 2 * NTILE], F32, isOutput=False)
    onesc = nc.declare_dram_parameter("onesc", [P, 1], F32, isOutput=False)        # ones column
    onesr = nc.declare_dram_parameter("onesr", [1, P], F32, isOutput=False)        # ones row (K=1 bcast)
    ident = nc.declare_dram_parameter("ident", [P, P], F32, isOutput=False)
    out = nc.declare_dram_parameter("out", [NT, ACTD], F32, isOutput=True)

    with TileContext(nc) as tc:
        with tc.tile_pool(name="pc", bufs=1) as pc:
            # ---- persistent SBUF tiles ----
            xrm_sb = pc.tile([P, NN], F32, tag="xrm")     # xrm[p, t*128+d] = xloc[t*128+p, d]
            xT_sb = pc.tile([P, NN], F32, tag="xT")       # xT[d, n]
            idx_sb = pc.tile([P, 4 * TTILE], I32, tag="idx")
            wA_sb = pc.tile([P, 4 * DIM], F32, tag="wA")
            wB_sb = pc.tile([P, 4 * DIM], F32, tag="wB")
            bs_sb = pc.tile([P, 4], F32, tag="bs")
            fcq_sb = pc.tile([P, DIM], F32, tag="fcq")
            fcr_sb = pc.tile([P, DIM], F32, tag="fcr")
            fcb_sb = pc.tile([P, 1], F32, tag="fcb")
            w1g_sb = pc.tile([P, DIM], F32, tag="w1g")
            w1x_sb = pc.tile([P, 4 * DIM], F32, tag="w1x")
            b1_sb = pc.tile([P, 1], F32, tag="b1")
            w2_sb = pc.tile([P, ACTD], F32, tag="w2")
            b2_sb = pc.tile([P, ACTD], F32, tag="b2")
            mask_sb = pc.tile([P, 2 * NTILE], F32, tag="mask")
            onc_sb = pc.tile([P, 1], F32, tag="onc")
            onr_sb = pc.tile([1, P], F32, tag="onr")
            id_sb = pc.tile([P, P], F32, tag="id")
            # Set2Set state (dim on partitions, graphs on free)
            hT = pc.tile([P, 2 * NTILE], F32, tag="hT")
            cT = pc.tile([P, 2 * NTILE], F32, tag="cT")
            rT = pc.tile([P, 2 * NTILE], F32, tag="rT")
            emask_sb = pc.tile([P, 2 * NTILE], F32, tag="emask")
            expe_sb = pc.tile([P, 2 * NTILE], F32, tag="expe")
            anm_sb = pc.tile([P, 2 * NTILE], F32, tag="anm")
            rs_sb = pc.tile([1, 2 * NTILE], F32, tag="rs")
            iS = pc.tile([P, 2 * NTILE], F32, tag="iS")
            fS = pc.tile([P, 2 * NTILE], F32, tag="fS")
            gTh = pc.tile([P, 2 * NTILE], F32, tag="gTh")
            oS = pc.tile([P, 2 * NTILE], F32, tag="oS")
            tnc = pc.tile([P, 2 * NTILE], F32, tag="tnc")
            geT_sb = pc.tile([P, 2 * NTILE], F32, tag="geT")
            hgT_sb = pc.tile([P, 2 * NTILE], F32, tag="hgT")

            # ---- load constants / inputs ----
            nc.sync.dma_start(out=xrm_sb[:].rearrange("p (t d) -> p t d", t=NTILE),
                              in_=xloc[:, :].rearrange("(t p) d -> p t d", p=P))
            nc.sync.dma_start(out=xT_sb[:], in_=xlocT[:, :])
            nc.sync.dma_start(out=idx_sb[:], in_=idx[:, :])
            nc.sync.dma_start(out=wA_sb[:], in_=wA[:, :])
            nc.sync.dma_start(out=wB_sb[:], in_=wB[:, :])
            nc.sync.dma_start(out=bs_sb[:], in_=bsum4[:, :])
            nc.sync.dma_start(out=fcq_sb[:], in_=fcwTq[:, :])
            nc.sync.dma_start(out=fcr_sb[:], in_=fcwTr[:, :])
            nc.sync.dma_start(out=fcb_sb[:], in_=fcb[:, :])
            nc.sync.dma_start(out=w1g_sb[:], in_=w1Tge[:, :])
            nc.sync.dma_start(out=w1x_sb[:].rearrange("p (s d) -> p s d", s=4),
                              in_=w1Tx[:, :].rearrange("(s p) d -> p s d", p=P))
            nc.sync.dma_start(out=b1_sb[:], in_=b1[:, :])
            nc.sync.dma_start(out=w2_sb[:], in_=w2T[:, :])
            nc.sync.dma_start(out=b2_sb[:], in_=b2t[:, :])
            nc.sync.dma_start(out=mask_sb[:], in_=maskc[:, :])
            nc.sync.dma_start(out=onc_sb[:], in_=onesc[:, :])
            nc.sync.dma_start(out=onr_sb[:], in_=onesr[:, :])
            nc.sync.dma_start(out=id_sb[:], in_=ident[:, :])
            nc.vector.memset(hT[:], 0.0)
            nc.vector.memset(cT[:], 0.0)
            nc.vector.memset(rT[:], 0.0)

            # ---- Set2Set: 6 steps ----
            with tc.tile_pool(name="pg", bufs=4, space="PSUM") as pg, \
                 tc.tile_pool(name="pe", bufs=1, space="PSUM") as ppe, \
                 tc.tile_pool(name="ps", bufs=1, space="PSUM") as pps, \
                 tc.tile_pool(name="pb", bufs=1, space="PSUM") as ppb, \
                 tc.tile_pool(name="pr", bufs=1, space="PSUM") as ppr:
                for step in range(STEPS):
                    # gates (transposed): gate k psum = wA[:,k].T @ hT + wB[:,k].T @ rT
                    gpsum = []
                    for k in range(4):
                        gp = pg.tile([P, 2 * NTILE], F32, tag="gates")
                        nc.tensor.matmul(out=gp[:], lhsT=wA_sb[:, k * P:(k + 1) * P],
                                         rhs=hT[:], start=True, stop=False)
                        nc.tensor.matmul(out=gp[:], lhsT=wB_sb[:, k * P:(k + 1) * P],
                                         rhs=rT[:], start=False, stop=True)
                        gpsum.append(gp)
                    # LSTM pointwise (i,f,g,o order); bias per-partition from bs_sb col k
                    nc.scalar.activation(out=iS[:], in_=gpsum[0][:], func=AF.Sigmoid,
                                         bias=bs_sb[:, 0:1])
                    nc.scalar.activation(out=fS[:], in_=gpsum[1][:], func=AF.Sigmoid,
                                         bias=bs_sb[:, 1:2])
                    nc.scalar.activation(out=gTh[:], in_=gpsum[2][:], func=AF.Tanh,
                                         bias=bs_sb[:, 2:3])
                    nc.scalar.activation(out=oS[:], in_=gpsum[3][:], func=AF.Sigmoid,
                                         bias=bs_sb[:, 3:4])
                    nc.vector.tensor_mul(out=cT[:], in0=fS[:], in1=cT[:])
                    nc.vector.tensor_mul(out=iS[:], in0=iS[:], in1=gTh[:])
                    nc.vector.tensor_add(out=cT[:], in0=cT[:], in1=iS[:])
                    nc.scalar.activation(out=tnc[:], in_=cT[:], func=AF.Tanh)
                    nc.vector.tensor_mul(out=hT[:], in0=oS[:], in1=tnc[:])  # hT = q

                    # e scores, node-major packed: pe[:, 2t+j] = x[128t+p] . q[2t+j]
                    pe = ppe.tile([P, 2 * NTILE], F32, tag="pe")
                    for t in range(NTILE):
                        nc.tensor.matmul(out=pe[:, 2 * t:2 * t + 2],
                                         lhsT=xT_sb[:, t * P:(t + 1) * P],
                                         rhs=hT[:, 2 * t:2 * t + 2],
                                         start=True, stop=True)
                    # mask invalid half, exp (no max-sub: |e| << 88)
                    nc.vector.tensor_add(out=emask_sb[:], in0=pe[:], in1=mask_sb[:])
                    nc.scalar.activation(out=expe_sb[:], in_=emask_sb[:], func=AF.Exp)
                    # segment sums: s[1, c] = sum_p expe[p, c]  (invalid half is exactly 0)
                    ps = pps.tile([1, 2 * NTILE], F32, tag="ps")
                    nc.tensor.matmul(out=ps[:], lhsT=onc_sb[:], rhs=expe_sb[:],
                                     start=True, stop=True)
                    nc.vector.reciprocal(out=rs_sb[:], in_=ps[:])
                    # broadcast 1/s across partitions via K=1 matmul
                    rsb = ppb.tile([P, 2 * NTILE], F32, tag="rsb")
                    nc.tensor.matmul(out=rsb[:], lhsT=onr_sb[:], rhs=rs_sb[:],
                                     start=True, stop=True)
                    nc.vector.tensor_mul(out=anm_sb[:], in0=expe_sb[:], in1=rsb[:])
                    # r (transposed, packed): pr[:, 2t+j] = sum_p a[p, 2t+j] * x[128t+p, :]
                    pr = ppr.tile([P, 2 * NTILE], F32, tag="pr")
                    for t in range(NTILE):
                        nc.tensor.matmul(out=pr[:, 2 * t:2 * t + 2],
                                         lhsT=xrm_sb[:, t * P:(t + 1) * P],
                                         rhs=anm_sb[:, 2 * t:2 * t + 2],
                                         start=True, stop=True)
                    nc.vector.tensor_copy(out=rT[:], in_=pr[:])

                # graph embed (transposed): geT = fc_w @ [q; r] + fc_b
                ge_ps = ppe.tile([P, 2 * NTILE], F32, tag="pe")
                nc.tensor.matmul(out=ge_ps[:], lhsT=fcq_sb[:], rhs=hT[:],
                                 start=True, stop=False)
                nc.tensor.matmul(out=ge_ps[:], lhsT=fcr_sb[:], rhs=rT[:],
                                 start=False, stop=True)
                nc.vector.tensor_scalar_add(out=geT_sb[:], in0=ge_ps[:],
                                            scalar1=fcb_sb[:, 0:1])
                # per-graph hidden contribution: hgT = W1_ge @ geT
                hg_ps = ppr.tile([P, 2 * NTILE], F32, tag="pr")
                nc.tensor.matmul(out=hg_ps[:], lhsT=w1g_sb[:], rhs=geT_sb[:],
                                 start=True, stop=True)
                nc.vector.tensor_copy(out=hgT_sb[:], in_=hg_ps[:])

            # ---- MLP over torsions: gather -> transpose -> matmuls ----
            with tc.tile_pool(name="pzs", bufs=6) as pzs, \
                 tc.tile_pool(name="pzt", bufs=4, space="PSUM") as pzt, \
                 tc.tile_pool(name="phd", bufs=2, space="PSUM") as phd, \
                 tc.tile_pool(name="plg", bufs=2, space="PSUM") as plg:
                for b in range(TTILE):
                    zts = []
                    for s in range(4):
                        col = s * TTILE + b
                        gat = pzs.tile([P, DIM], F32, tag="gat")
                        nc.gpsimd.indirect_dma_start(
                            out=gat[:], out_offset=None,
                            in_=xfull[:, :],
                            in_offset=bass.IndirectOffsetOnAxis(
                                ap=idx_sb[:, col:col + 1], axis=0),
                        )
                        ztp = pzt.tile([P, P], F32, tag="ztp")
                        nc.tensor.transpose(out=ztp[:], in_=gat[:], identity=id_sb[:])
                        zt = pzs.tile([P, P], F32, tag="zt")
                        if s % 2 == 0:
                            nc.vector.tensor_copy(out=zt[:], in_=ztp[:])
                        else:
                            nc.scalar.copy(out=zt[:], in_=ztp[:])
                        zts.append(zt)
                    hd = phd.tile([P, P], F32, tag="hd")
                    for s in range(4):
                        nc.tensor.matmul(out=hd[:], lhsT=w1x_sb[:, s * P:(s + 1) * P],
                                         rhs=zts[s][:], start=(s == 0), stop=(s == 3))
                    # + per-graph term (broadcast 4 graphs x 32) then relu(. + b1)
                    hsl = hgT_sb[:, b * 4:(b + 1) * 4]
                    hbc = bass.AP(hsl.tensor, hsl.offset, list(hsl.ap) + [[0, MAX_T]])
                    hdn = pzs.tile([P, P], F32, tag="hdn")
                    nc.vector.tensor_add(
                        out=hdn[:].rearrange("p (g u) -> p g u", g=4),
                        in0=hd[:].rearrange("p (g u) -> p g u", g=4),
                        in1=hbc)
                    nc.scalar.activation(out=hdn[:], in_=hdn[:], func=AF.Relu,
                                         bias=b1_sb[:, 0:1])
                    # logits (row-major out): [128 t, 36]
                    lg = plg.tile([P, ACTD], F32, tag="lg")
                    nc.tensor.matmul(out=lg[:], lhsT=hdn[:], rhs=w2_sb[:],
                                     start=True, stop=True)
                    lsb = pzs.tile([P, ACTD], F32, tag="lsb")
                    nc.vector.tensor_add(out=lsb[:], in0=lg[:], in1=b2_sb[:])
                    nc.sync.dma_start(out=out[b * P:(b + 1) * P, :], in_=lsb[:])
    nc.compile()
    return nc


def _host_prep(inputs):
    x = np.ascontiguousarray(inputs["x"], np.float32)
    nonring = np.asarray(inputs["nonring"], np.int32)
    w_ih = np.asarray(inputs["w_ih"], np.float32)
    w_hh = np.asarray(inputs["w_hh"], np.float32)
    b_ih = np.asarray(inputs["b_ih"], np.float32)
    b_hh = np.asarray(inputs["b_hh"], np.float32)
    fc_w = np.asarray(inputs["fc_w"], np.float32)
    fc_b = np.asarray(inputs["fc_b"], np.float32)
    mlp_w1 = np.asarray(inputs["mlp_w1"], np.float32)
    mlp_b1 = np.asarray(inputs["mlp_b1"], np.float32)
    mlp_w2 = np.asarray(inputs["mlp_w2"], np.float32)
    mlp_b2 = np.asarray(inputs["mlp_b2"], np.float32)

    # replicated small tensors
    wA = np.ascontiguousarray((w_ih[:, :DIM] + w_hh).T)             # [128, 512]
    wB = np.ascontiguousarray(w_ih[:, DIM:].T)                      # [128, 512]
    bsum4 = np.ascontiguousarray((b_ih + b_hh).reshape(4, P).T)     # [128, 4]
    fcwT = fc_w.T                                                   # [256, 128]
    w1T = mlp_w1.T                                                  # [640, 128]
    rep = {
        "xfull": x,
        "wA": wA, "wB": wB, "bsum4": bsum4,
        "fcwTq": np.ascontiguousarray(fcwT[:DIM]),
        "fcwTr": np.ascontiguousarray(fcwT[DIM:]),
        "fcb": np.ascontiguousarray(fc_b.reshape(P, 1)),
        "w1Tge": np.ascontiguousarray(w1T[:DIM]),
        "w1Tx": np.ascontiguousarray(w1T[DIM:]),
        "b1": np.ascontiguousarray(mlp_b1.reshape(P, 1)),
        "w2T": np.ascontiguousarray(mlp_w2.T),
        "b2t": np.ascontiguousarray(np.tile(mlp_b2.reshape(1, ACTD), (P, 1))),
        "onesc": np.ones((P, 1), np.float32),
        "onesr": np.ones((1, P), np.float32),
        "ident": np.eye(P, dtype=np.float32),
    }
    # mask: valid iff (p < 64) == (col even)
    p = np.arange(P).reshape(P, 1)
    c = np.arange(2 * NTILE).reshape(1, 2 * NTILE)
    valid = (p < NODES_PER_G) == (c % 2 == 0)
    rep["maskc"] = np.where(valid, 0.0, -1e30).astype(np.float32)

    in_maps = []
    for k in range(NC):
        xl = x[k * NN:(k + 1) * NN]
        nr = nonring[k * NT:(k + 1) * NT]                  # [8192, 4]
        flat = nr.T.reshape(-1)                            # slot-major [32768]
        idx = np.ascontiguousarray(flat.reshape(4 * TTILE, P).T)  # [128, 256]
        m = dict(rep)
        m["xloc"] = np.ascontiguousarray(xl)
        m["xlocT"] = np.ascontiguousarray(xl.T)
        m["idx"] = idx.astype(np.int32)
        in_maps.append(m)
    return in_maps


def kernel(**inputs) -> np.ndarray:
    global LAST, _CACHED
    if _CACHED is None:
        _CACHED = build_bass()
    nc = _CACHED
    in_maps = _host_prep(inputs)
    LAST = run_bass_kernel_spmd(nc, in_maps, core_ids=list(range(NC)))
    outs = [LAST.results[k]["out"].reshape(NG, MAX_T, ACTD) for k in range(NC)]
    return np.concatenate(outs, axis=0)


if __name__ == "__main__":
    nc = build_bass()
    print("build ok")

